# revision 22
# baseline (speedup 1.0000x reference)
"""Multi-head attention (b=2, s=2048, h=1024, 16 heads x 64) on 8 NeuronCores.

Sharding: tensor-parallel over heads. Core c owns heads {2c, 2c+1} for both
batches. Each core computes a full [4096, 1024] partial of the output
projection (scaled by 1024); the host sums the partials, divides by 1024, and
adds bias corrections (b_out + b_v @ w_out; k-bias dropped: softmax invariant).

All matmuls run in fp8e4m3 DoubleRow perf mode (0.5 cyc/row, 2 k-subtiles per
instruction). Power-of-2 scale management keeps every fp8 tensor in e4m3's
normal range:
  x*8, w_qkv*64, w_out*64 quantized on host.
  Q = psum*2^-5 + 16*bq -> fp8 ; K,V = psum*2^-5 -> fp8  (DVE evac)
  S_psum = 256*(q.k);  P = exp(2^-11 * S_psum) -> fp8    (true logits = qk/8)
  O = sum P*V (psum, col 64 = sum P via ones column in V)
  onat = O * (1/sums) -> bf16  (= 16*attn_out)
  osT  = transpose(onat) -> fp8  (PE transpose via bf16 psum)
  y_psum = osT^T @ (64*w_out) = 1024*y ; evac bf16 ; host /1024.

Dataflow per (batch, qgroup, head) "block" (16 blocks):
  S^T tiles [128 kt, 512 q] via 1 DR matmul each (K=64, zero 2nd subtile),
  exp on ScalarE ([128,2,512] psum -> fp8 P^T in SBUF), some tiles relayed
  DVE-copy -> GPSIMD pow(e^c, S) to offload ScalarE.
  AV in natural orientation: O[128 q, 65] += P^T-slice^T @ V-slice, 8 DR
  chunks, 4 q-subtiles sequentially (single psum bank each).
  Out-projection: 1 DR matmul per [128 q, 512 cols] (both heads contracted).

Engines: ScalarE ~ exp + share of out-evacs; DVE ~ evacs/recip/relay; Pool ~
pow-exp relay + memsets; PE ~50% idle (fp8 made it cheap). The elementwise
engines are the bottleneck, so evac engine assignments are knob-tunable.
"""

import contextlib
import sys
from collections import deque

import numpy as np

sys.path.insert(0, "/opt/trn_rl_repo")

import ml_dtypes  # noqa: E402

import concourse.tile as tile  # noqa: E402
from concourse import bacc, mybir  # noqa: E402
from concourse.bass_utils import run_bass_kernel_spmd  # noqa: E402
from concourse.masks import make_identity  # noqa: E402

BF16 = mybir.dt.bfloat16
F32 = mybir.dt.float32
FP8 = mybir.dt.float8e4
AF = mybir.ActivationFunctionType
DR = mybir.MatmulPerfMode.DoubleRow
f8np = ml_dtypes.float8_e4m3
bfnp = ml_dtypes.bfloat16

B = 2
S = 2048
T = B * S
H = 1024
N_CORES = 8

EXP_SCALE = 2.0 ** -11          # exp(EXP_SCALE * S_psum) = exp(true logits)
QKV_EVAC = 2.0 ** -5            # psum -> fp8 scale for q/k/v
OUT_DIV = 1024.0                # host divides partials by this

# ---- scheduling knobs ----
RELAY_MOD = 8                   # exp idx % RELAY_MOD == RELAY_PHASE -> pool
RELAY_PHASE = 4
OP_EVAC_PAT = "v"               # engine per OP evac: a=ScalarE v=DVE
QK_EVAC_PAT = "v"
V_EVAC_PAT = "v"
ONAT_PAT = "v"
OST_PAT = "v"

_program_cache = {}


class Ctx:
    pass


class Gen:
    """Generator wrapper with exhaustion flag."""

    def __init__(self, gen):
        self.gen = gen
        self.done = False

    def step(self):
        if self.done:
            return False
        try:
            next(self.gen)
            return True
        except StopIteration:
            self.done = True
            return False


class Filler:
    """FIFO of Gen wrappers; pull() advances the head generator one unit."""

    def __init__(self):
        self.q = deque()

    def add(self, gen):
        g = Gen(gen) if not isinstance(gen, Gen) else gen
        self.q.append(g)
        return g

    def add_front(self, gen):
        g = Gen(gen) if not isinstance(gen, Gen) else gen
        self.q.appendleft(g)
        return g

    def pull(self, n=1):
        while n > 0 and self.q:
            if self.q[0].step():
                n -= 1
            else:
                self.q.popleft()

    def drain_until(self, g):
        """Drain from the head until generator g is exhausted (g must be in
        the queue; everything ahead of it drains fully)."""
        while not g.done:
            if not self.q:
                raise RuntimeError("drain_until: generator not in queue")
            self.pull(1)

    def drain(self):
        while self.q:
            self.pull()


def _pick(pat, idx):
    return pat[idx % len(pat)]


def copy_engine(nc, which):
    if which == "a":
        return lambda o, i: nc.scalar.copy(o, i)
    return lambda o, i: nc.vector.tensor_copy(o, i)


# --------------------------------------------------------------------------
# stage A: QKV projections (fillers)
# --------------------------------------------------------------------------

def q_unit(nc, c, b, t0=None, t1=None, g=None):
    """Q^T projection+evac for a token range (default one 512-token group)."""
    if g is not None:
        t0, t1 = g * 512, (g + 1) * 512
    psq = c.psM.tile([128, t1 - t0], F32, tag="mm", name=f"psq{b}{t0}")
    for o in range(4):
        nc.tensor.matmul(
            psq[:], c.wq_sb[:, 2 * o:2 * o + 2, :],
            c.xt_sb[:, 2 * o:2 * o + 2, b * 2048 + t0:b * 2048 + t1],
            start=(o == 0), stop=(o == 3), perf_mode=DR,
        )
    nc.vector.tensor_scalar(
        c.qt8[b][:, 0, t0:t1], psq[:], QKV_EVAC, c.bq_sb[:],
        op0=mybir.AluOpType.mult, op1=mybir.AluOpType.add,
    )
    yield


def k_unit(nc, c, b, t0=None, t1=None, g=None):
    if g is not None:
        t0, t1 = g * 512, (g + 1) * 512
    psk = c.psM.tile([128, t1 - t0], F32, tag="mm", name=f"psk{b}{t0}")
    for o in range(4):
        nc.tensor.matmul(
            psk[:], c.wk_sb[:, 2 * o:2 * o + 2, :],
            c.xt_sb[:, 2 * o:2 * o + 2, b * 2048 + t0:b * 2048 + t1],
            start=(o == 0), stop=(o == 3), perf_mode=DR,
        )
    nc.vector.tensor_scalar(
        c.kt8[b][:, 0, t0:t1], psk[:], QKV_EVAC, None, op0=mybir.AluOpType.mult,
    )
    yield


def v_units(nc, c, b):
    """V natural [token, vcol] per 128-token tile; v8 layout [128,16,2,65]."""
    for t in range(16):
        psv = c.psM.tile([128, 128], F32, tag="mm", name=f"psv{b}{t}")
        for o in range(4):
            nc.tensor.matmul(
                psv[:], c.xt_sb[:, 2 * o:2 * o + 2,
                b * 2048 + t * 128:b * 2048 + (t + 1) * 128],
                c.wv_sb[:, 2 * o:2 * o + 2, :],
                start=(o == 0), stop=(o == 3), perf_mode=DR,
            )
        nc.vector.tensor_scalar(
            c.v8[b][:, t, :, 0:64],
            psv[:].rearrange("p (h x) -> p h x", h=2),
            QKV_EVAC, None, op0=mybir.AluOpType.mult,
        )
        yield


# --------------------------------------------------------------------------
# attention block: S + exp (backbone), AV + epilogue + OP (fillers)
# --------------------------------------------------------------------------

def emit_s_exp(nc, c, b, qg, h, g2):
    """Two S^T kt tiles + exp -> fp8 P^T tile [128, 2, 512].

    ScalarE path: S pair into one psS [128,2,512] tile, one Exp activation.
    Relay path (every RELAY_MOD-th): S pair into two psM [128,512] tiles
    (keeps the psS ring private to the ScalarE exp chain), DVE-copy to bf16,
    pow(e^c, S) on GPSIMD.
    """
    q0 = qg * 512
    hs = slice(h * 64, (h + 1) * 64)
    name = f"{b}{qg}{h}{g2}"
    idx = c.exp_idx
    c.exp_idx += 1
    pT = c.ptp.tile([128, 2, 512], FP8, tag="pT", name=f"pT{name}")
    relay = idx % RELAY_MOD == RELAY_PHASE

    def s_mm(out, j):
        kt = g2 * 2 + j
        nc.tensor.matmul(
            out,
            c.kt8[b][hs, :, kt * 128:(kt + 1) * 128],
            c.qt8[b][hs, :, q0:q0 + 512],
            start=True, stop=True, perf_mode=DR, tile_position=(h * 64, 0),
        )

    if relay:
        scop = c.work.tile([128, 2, 512], F32, tag="scop", bufs=3,
                           name=f"scop{name}")
        for j in range(2):
            psj = c.psM.tile([128, 512], F32, tag="mm", name=f"psr{name}{j}")
            s_mm(psj[:], j)
            nc.vector.tensor_copy(scop[:, j, :], psj[:])
        nc.gpsimd.tensor_tensor(pT[:], c.econ[:], scop[:], mybir.AluOpType.pow)
    else:
        ps2 = c.psS.tile([128, 2, 512], F32, tag="s2", name=f"ps2_{name}")
        for j in range(2):
            s_mm(ps2[:, j, :], j)
        nc.scalar.activation(pT[:], ps2[:], AF.Exp, scale=EXP_SCALE)
    return pT


def av_epilogue(nc, c, b, qg, h, pts, osts, tail=False):
    """AV + normalize + transpose for block (b,qg,h); one qsub at a time.

    tail=True: spread the copies/scales across ScalarE too (kernel drain,
    ScalarE otherwise idle)."""
    for qsub in range(4):
        po = c.psO.tile([128, 512], F32, tag="acc", name=f"po{b}{qg}{h}{qsub}")
        qs = slice(qsub * 128, (qsub + 1) * 128)
        for g2 in range(8):
            nc.tensor.matmul(
                po[:, 0:65],
                pts[g2][:, :, qs],
                c.v8[b][:, 2 * g2:2 * g2 + 2, h, :],
                start=(g2 == 0), stop=(g2 == 7), perf_mode=DR,
            )
        yield
        recip = c.work.tile([128, 1], F32, tag="recip", bufs=4,
                            name=f"rc{b}{qg}{h}{qsub}")
        nc.vector.reciprocal(recip[:], po[:, 64:65])
        onat = c.work.tile([128, 64], BF16, tag="onat", bufs=4,
                           name=f"on{b}{qg}{h}{qsub}")
        if tail and qsub % 2 == 0:
            nc.scalar.activation(onat[:], po[:, 0:64], AF.Identity,
                                 scale=recip[:])
        else:
            nc.vector.tensor_scalar(
                onat[:], po[:, 0:64], recip[:], None, op0=mybir.AluOpType.mult,
            )
        tr = c.psO.tile([128, 128], BF16, tag="acc", name=f"tr{b}{qg}{h}{qsub}")
        nc.tensor.transpose(tr[0:64, :], onat[:], c.ident[:])
        ost_eng = ("a" if qsub % 2 else "v") if tail else _pick(OST_PAT, c.ost_idx)
        copy_engine(nc, ost_eng)(osts[qsub][0:64, h, :], tr[0:64, :])
        c.ost_idx += 1
        yield


def op_unit(nc, c, b, qg, osts, tail=False):
    """Out-projection for one (b, qg): 4 qsubs x [128, 1024]."""
    for qsub in range(4):
        gq = b * 2048 + qg * 512 + qsub * 128
        ob = c.work.tile([128, 1024], BF16, tag="ob", bufs=3,
                         name=f"ob{b}{qg}{qsub}")
        for n in range(2):
            psy = c.psM.tile([128, 512], F32, tag="mm", name=f"psy{b}{qg}{qsub}{n}")
            nc.tensor.matmul(
                psy[:], osts[qsub][0:64, :, :],
                c.wo_sb[0:64, :, n * 512:(n + 1) * 512],
                start=True, stop=True, perf_mode=DR,
            )
            eng = ("a" if n else "v") if tail else _pick(OP_EVAC_PAT, c.op_idx)
            copy_engine(nc, eng)(ob[:, n * 512:(n + 1) * 512], psy[:])
            c.op_idx += 1
            if tail:
                nc.sync.dma_start(
                    c.out[gq:gq + 128, n * 512:(n + 1) * 512],
                    ob[:, n * 512:(n + 1) * 512])
        if not tail:
            nc.sync.dma_start(c.out[gq:gq + 128, :], ob[:])
        yield


# --------------------------------------------------------------------------

def build_body(tc, xt, wq, wk, wv, bq, wo, out):
    nc = tc.nc
    c = Ctx()
    c.out = out
    c.exp_idx = 0
    c.op_idx = 0
    c.ost_idx = 0
    c.ost_cur = [None] * 4
    c.ost_prev = [None] * 4
    with contextlib.ExitStack() as ctx:
        c.const = ctx.enter_context(tc.tile_pool(name="const", bufs=1))
        c.work = ctx.enter_context(tc.tile_pool(name="work", bufs=3))
        c.ptp = ctx.enter_context(tc.tile_pool(name="ptile", bufs=16))
        # PSUM (8 banks): s2 [128,2,512]f32 x2 = 4, acc(+tr) x2 = 2, mm x2 = 2
        c.psS = ctx.enter_context(tc.tile_pool(name="psS", bufs=2, space="PSUM"))
        c.psO = ctx.enter_context(tc.tile_pool(name="psO", bufs=2, space="PSUM"))
        c.psM = ctx.enter_context(tc.tile_pool(name="psM", bufs=2, space="PSUM"))

        # ---- DMA in consumption order ----
        c.wq_sb = c.const.tile([128, 8, 128], FP8, name="wq_sb")
        nc.sync.dma_start(c.wq_sb[:], wq[:])
        c.bq_sb = c.const.tile([128, 1], F32, name="bq_sb")
        nc.sync.dma_start(c.bq_sb[:], bq[:])
        actwarm = c.work.tile([1, 1], F32, tag="actwarm", bufs=1, name="actwarm")
        nc.scalar.activation(actwarm[:], c.bq_sb[0:1, 0:1], AF.Exp)

        c.xt_sb = c.const.tile([128, 8, T], FP8, name="xt_sb")

        def load_xt(t0, t1):
            nc.sync.dma_start(c.xt_sb[:, :, t0:t1], xt[:, :, t0:t1])

        c.wk_sb = c.const.tile([128, 8, 128], FP8, name="wk_sb")
        nc.sync.dma_start(c.wk_sb[:], wk[:])
        load_xt(0, 256)
        load_xt(256, 512)
        c.wv_sb = c.const.tile([128, 8, 128], FP8, name="wv_sb")
        nc.sync.dma_start(c.wv_sb[:], wv[:])
        load_xt(512, 1024)
        load_xt(1024, 2048)
        c.wo_sb = c.const.tile([64, 2, 1024], FP8, name="wo_sb")
        nc.sync.dma_start(c.wo_sb[:], wo[:])
        load_xt(2048, 3072)
        load_xt(3072, 4096)

        c.ident = c.const.tile([128, 128], BF16, name="ident")
        make_identity(nc, c.ident[:])
        c.econ = c.const.tile([128, 2, 512], F32, name="econ")

        # per-batch fp8 operand tensors
        c.qt8 = [c.const.tile([128, 2, S], FP8, name=f"qt8_{b}") for b in range(2)]
        c.kt8 = [c.const.tile([128, 2, S], FP8, name=f"kt8_{b}") for b in range(2)]
        c.v8 = [c.const.tile([128, 16, 2, 65], FP8, name=f"v8_{b}") for b in range(2)]
        # zero the second DR subtile of q/k (0 * finite = 0 in the S matmul).
        # Chunked on Pool in S-consumption order so the first S-pairs don't
        # wait for full-tensor memsets.
        nc.gpsimd.memset(c.kt8[0][:, 1, 0:512], 0.0)
        nc.gpsimd.memset(c.qt8[0][:, 1, 0:512], 0.0)
        nc.gpsimd.memset(c.econ[:], float(np.exp(EXP_SCALE)))
        nc.gpsimd.memset(c.kt8[0][:, 1, 512:2048], 0.0)
        nc.gpsimd.memset(c.qt8[0][:, 1, 512:2048], 0.0)
        for b in range(2):
            nc.vector.memset(c.v8[b][:, :, :, 64:65], 1.0)  # softmax-sum ones
        nc.gpsimd.memset(c.kt8[1][:, 1, :], 0.0)
        nc.gpsimd.memset(c.qt8[1][:, 1, :], 0.0)

        # ---- emission ----
        fill = Filler()
        # prefill: K0, Q0 of batch 0 up-front, fine-grained to chase the
        # first xt DMA chunks (K0a only needs tokens 0:256)
        for rng in ((k_unit, 0, 256), (q_unit, 0, 512), (k_unit, 256, 512)):
            for _ in rng[0](nc, c, 0, rng[1], rng[2]):
                pass
        k_gens = {0: [], 1: []}
        q_gens = {}
        v_gens = {}
        for g in range(1, 4):
            k_gens[0].append(fill.add(k_unit(nc, c, 0, g=g)))
        v_gens[0] = fill.add(v_units(nc, c, 0))
        for g in range(1, 4):
            q_gens[(0, g)] = fill.add(q_unit(nc, c, 0, g=g))
        for g in range(4):
            k_gens[1].append(fill.add(k_unit(nc, c, 1, g=g)))
        q_gens[(1, 0)] = fill.add(q_unit(nc, c, 1, g=0))
        v_gens[1] = fill.add(v_units(nc, c, 1))
        for g in range(1, 4):
            q_gens[(1, g)] = fill.add(q_unit(nc, c, 1, g=g))

        blocks = [(b, qg, h) for b in range(2) for qg in range(4) for h in range(2)]
        osts_map = {}

        def osts_for(b, qg):
            if (b, qg) not in osts_map:
                osts_map[(b, qg)] = [
                    c.work.tile([64, 2, 128], FP8, tag=f"osT{q}", bufs=2,
                                name=f"osT{b}{qg}{q}")
                    for q in range(4)
                ]
            return osts_map[(b, qg)]

        pts_prev = None
        prev_blk = None
        for n, (b, qg, h) in enumerate(blocks):
            # emission-order prerequisites: the epilogue of prev_blk reads
            # v8[prev_b]; this block's S matmuls read kt8/qt8 slices.
            if prev_blk is not None:
                fill.drain_until(v_gens[prev_blk[0]])
            for kg in k_gens[b]:
                fill.drain_until(kg)
            if (b, qg) in q_gens:
                fill.drain_until(q_gens[(b, qg)])
            pts = []
            if prev_blk is not None:
                fill.add_front(av_epilogue(nc, c, *prev_blk, pts_prev,
                                           osts_for(prev_blk[0], prev_blk[1])))
            for g2 in range(8):
                pts.append(emit_s_exp(nc, c, b, qg, h, g2))
                fill.pull(2)
            if prev_blk is not None and prev_blk[2] == 1:
                # both heads of (prev b, prev qg) done once its epilogue runs
                fill.add(op_unit(nc, c, prev_blk[0], prev_blk[1],
                                 osts_for(prev_blk[0], prev_blk[1])))
            pts_prev = pts
            prev_blk = (b, qg, h)
        fill.add_front(av_epilogue(nc, c, *prev_blk, pts_prev,
                                   osts_for(prev_blk[0], prev_blk[1]),
                                   tail=True))
        fill.add(op_unit(nc, c, prev_blk[0], prev_blk[1],
                         osts_for(prev_blk[0], prev_blk[1]), tail=True))
        fill.drain()


def build_program():
    key = (RELAY_MOD, RELAY_PHASE, OP_EVAC_PAT, QK_EVAC_PAT, V_EVAC_PAT,
           ONAT_PAT, OST_PAT)
    if key in _program_cache:
        return _program_cache[key]
    nc = bacc.Bacc("TRN2", target_bir_lowering=False, debug=False)
    xt = nc.dram_tensor("xt", [128, 8, T], FP8, kind="ExternalInput").ap()
    wq = nc.dram_tensor("wq", [128, 8, 128], FP8, kind="ExternalInput").ap()
    wk = nc.dram_tensor("wk", [128, 8, 128], FP8, kind="ExternalInput").ap()
    wv = nc.dram_tensor("wv", [128, 8, 128], FP8, kind="ExternalInput").ap()
    bq = nc.dram_tensor("bq", [128, 1], F32, kind="ExternalInput").ap()
    wo = nc.dram_tensor("wo", [64, 2, 1024], FP8, kind="ExternalInput").ap()
    out = nc.dram_tensor("out", [T, H], BF16, kind="ExternalOutput").ap()
    with tile.TileContext(nc) as tc:
        build_body(tc, xt, wq, wk, wv, bq, wo, out)
    nc.compile()
    _program_cache[key] = nc
    return nc


def make_in_maps(x, w_qkv, b_qkv, w_out):
    x = np.asarray(x, dtype=np.float32)
    w_qkv = np.asarray(w_qkv, dtype=np.float32)
    b_qkv = np.asarray(b_qkv, dtype=np.float32)
    w_out = np.asarray(w_out, dtype=np.float32)

    # x^T [H, T] scaled by 8, in [128, 8, T] layout (hidden ktile on dim1)
    xt = np.ascontiguousarray(
        (x.reshape(T, H).T * 8.0).reshape(8, 128, T).transpose(1, 0, 2)
    ).astype(f8np)

    def prep_w(w):
        # [1024 hidden, 128 cols] -> [128 part, 8 ktile, 128 col], *64
        return np.ascontiguousarray(
            (w * 64.0).reshape(8, 128, 128).transpose(1, 0, 2)
        ).astype(f8np)

    in_maps = []
    for cc in range(N_CORES):
        sl = slice(cc * 128, (cc + 1) * 128)
        wo_c = np.ascontiguousarray(
            (w_out[sl, :] * 64.0).reshape(2, 64, H).transpose(1, 0, 2)
        ).astype(f8np)
        in_maps.append({
            "xt": xt,
            "wq": prep_w(w_qkv[:, sl]),
            "wk": prep_w(w_qkv[:, H + cc * 128:H + (cc + 1) * 128]),
            "wv": prep_w(w_qkv[:, 2 * H + cc * 128:2 * H + (cc + 1) * 128]),
            "bq": (b_qkv[sl] * 16.0).astype(np.float32).reshape(128, 1),
            "wo": wo_c,
        })
    return in_maps


def finalize(results, b_qkv, b_out, w_out):
    b_qkv = np.asarray(b_qkv, dtype=np.float32)
    b_out = np.asarray(b_out, dtype=np.float32)
    w_out = np.asarray(w_out, dtype=np.float32)
    acc = np.zeros((T, H), np.float32)
    for r in results:
        acc += np.asarray(r["out"], dtype=np.float32)
    acc /= OUT_DIV
    corr = b_out + b_qkv[2 * H:] @ w_out
    return (acc + corr).reshape(B, S, H).astype(np.float32)


def kernel(x, w_qkv, b_qkv, w_out, b_out):
    import os

    os.environ["BASS_NEVER_TRACE"] = "1"
    nc = build_program()
    in_maps = make_in_maps(x, w_qkv, b_qkv, w_out)
    res = run_bass_kernel_spmd(nc, in_maps, list(range(N_CORES)))
    return finalize(res.results, b_qkv, b_out, w_out)


# revision 24
# speedup vs baseline: 1.0185x; 1.0185x over previous
"""Multi-head attention (b=2, s=2048, h=1024, 16 heads x 64) on 8 NeuronCores.

Sharding: tensor-parallel over heads. Core c owns heads {2c, 2c+1} for both
batches. Each core computes a full [4096, 1024] partial of the output
projection (scaled by 1024); the host sums the partials, divides by 1024, and
adds bias corrections (b_out + b_v @ w_out; k-bias dropped: softmax invariant).

All matmuls run in fp8e4m3 DoubleRow perf mode (0.5 cyc/row, 2 k-subtiles per
instruction). Power-of-2 scale management keeps every fp8 tensor in e4m3's
normal range:
  x*8, w_qkv*64, w_out*64 quantized on host.
  Q = psum*2^-5 + 16*bq -> fp8 ; K,V = psum*2^-5 -> fp8  (DVE evac)
  S_psum = 256*(q.k);  P = exp(2^-11 * S_psum) -> fp8    (true logits = qk/8)
  O = sum P*V (psum, col 64 = sum P via ones column in V)
  onat = O * (1/sums) -> bf16  (= 16*attn_out)
  osT  = transpose(onat) -> fp8  (PE transpose via bf16 psum)
  y_psum = osT^T @ (64*w_out) = 1024*y ; evac bf16 ; host /1024.

Dataflow per (batch, qgroup, head) "block" (16 blocks):
  S^T tiles [128 kt, 512 q] via 1 DR matmul each (K=64, zero 2nd subtile),
  exp on ScalarE ([128,2,512] psum -> fp8 P^T in SBUF), some tiles relayed
  DVE-copy -> GPSIMD pow(e^c, S) to offload ScalarE.
  AV in natural orientation: O[128 q, 65] += P^T-slice^T @ V-slice, 8 DR
  chunks, 4 q-subtiles sequentially (single psum bank each).
  Out-projection: 1 DR matmul per [128 q, 512 cols] (both heads contracted).

Engines: ScalarE ~ exp + share of out-evacs; DVE ~ evacs/recip/relay; Pool ~
pow-exp relay + memsets; PE ~50% idle (fp8 made it cheap). The elementwise
engines are the bottleneck, so evac engine assignments are knob-tunable.
"""

import contextlib
import sys
from collections import deque

import numpy as np

sys.path.insert(0, "/opt/trn_rl_repo")

import ml_dtypes  # noqa: E402

import concourse.tile as tile  # noqa: E402
from concourse import bacc, mybir  # noqa: E402
from concourse.bass_utils import run_bass_kernel_spmd  # noqa: E402
from concourse.masks import make_identity  # noqa: E402

BF16 = mybir.dt.bfloat16
F32 = mybir.dt.float32
FP8 = mybir.dt.float8e4
AF = mybir.ActivationFunctionType
DR = mybir.MatmulPerfMode.DoubleRow
f8np = ml_dtypes.float8_e4m3
bfnp = ml_dtypes.bfloat16

B = 2
S = 2048
T = B * S
H = 1024
N_CORES = 8

EXP_SCALE = 2.0 ** -11          # exp(EXP_SCALE * S_psum) = exp(true logits)
QKV_EVAC = 2.0 ** -5            # psum -> fp8 scale for q/k/v
OUT_DIV = 1024.0                # host divides partials by this

# ---- scheduling knobs ----
RELAY_MOD = 8                   # exp idx % RELAY_MOD == RELAY_PHASE -> pool
RELAY_PHASE = 4
OP_EVAC_PAT = "v"               # engine per OP evac: a=ScalarE v=DVE
QK_EVAC_PAT = "v"
V_EVAC_PAT = "v"
ONAT_PAT = "v"
OST_PAT = "v"

_program_cache = {}


class Ctx:
    pass


class Gen:
    """Generator wrapper with exhaustion flag."""

    def __init__(self, gen):
        self.gen = gen
        self.done = False

    def step(self):
        if self.done:
            return False
        try:
            next(self.gen)
            return True
        except StopIteration:
            self.done = True
            return False


class Filler:
    """FIFO of Gen wrappers; pull() advances the head generator one unit."""

    def __init__(self):
        self.q = deque()

    def add(self, gen):
        g = Gen(gen) if not isinstance(gen, Gen) else gen
        self.q.append(g)
        return g

    def add_front(self, gen):
        g = Gen(gen) if not isinstance(gen, Gen) else gen
        self.q.appendleft(g)
        return g

    def pull(self, n=1):
        while n > 0 and self.q:
            if self.q[0].step():
                n -= 1
            else:
                self.q.popleft()

    def drain_until(self, g):
        """Drain from the head until generator g is exhausted (g must be in
        the queue; everything ahead of it drains fully)."""
        while not g.done:
            if not self.q:
                raise RuntimeError("drain_until: generator not in queue")
            self.pull(1)

    def drain(self):
        while self.q:
            self.pull()


def _pick(pat, idx):
    return pat[idx % len(pat)]


def copy_engine(nc, which):
    if which == "a":
        return lambda o, i: nc.scalar.copy(o, i)
    return lambda o, i: nc.vector.tensor_copy(o, i)


# --------------------------------------------------------------------------
# stage A: QKV projections (fillers)
# --------------------------------------------------------------------------

def q_unit(nc, c, b, t0=None, t1=None, g=None):
    """Q^T projection+evac for a token range (default one 512-token group)."""
    if g is not None:
        t0, t1 = g * 512, (g + 1) * 512
    psq = c.psM.tile([128, t1 - t0], F32, tag="mm", name=f"psq{b}{t0}")
    for o in range(4):
        nc.tensor.matmul(
            psq[:], c.wq_sb[:, 2 * o:2 * o + 2, :],
            c.xt_sb[:, 2 * o:2 * o + 2, b * 2048 + t0:b * 2048 + t1],
            start=(o == 0), stop=(o == 3), perf_mode=DR,
        )
    nc.vector.tensor_scalar(
        c.qt8[b][:, 0, t0:t1], psq[:], QKV_EVAC, c.bq_sb[:],
        op0=mybir.AluOpType.mult, op1=mybir.AluOpType.add,
    )
    yield


def k_unit(nc, c, b, t0=None, t1=None, g=None):
    if g is not None:
        t0, t1 = g * 512, (g + 1) * 512
    psk = c.psM.tile([128, t1 - t0], F32, tag="mm", name=f"psk{b}{t0}")
    for o in range(4):
        nc.tensor.matmul(
            psk[:], c.wk_sb[:, 2 * o:2 * o + 2, :],
            c.xt_sb[:, 2 * o:2 * o + 2, b * 2048 + t0:b * 2048 + t1],
            start=(o == 0), stop=(o == 3), perf_mode=DR,
        )
    nc.vector.tensor_scalar(
        c.kt8[b][:, 0, t0:t1], psk[:], QKV_EVAC, None, op0=mybir.AluOpType.mult,
    )
    yield


def v_units(nc, c, b):
    """V natural [token, vcol] per 128-token tile; v8 layout [128,16,2,65]."""
    for t in range(16):
        psv = c.psM.tile([128, 128], F32, tag="mm", name=f"psv{b}{t}")
        for o in range(4):
            nc.tensor.matmul(
                psv[:], c.xt_sb[:, 2 * o:2 * o + 2,
                b * 2048 + t * 128:b * 2048 + (t + 1) * 128],
                c.wv_sb[:, 2 * o:2 * o + 2, :],
                start=(o == 0), stop=(o == 3), perf_mode=DR,
            )
        nc.vector.tensor_scalar(
            c.v8[b][:, t, :, 0:64],
            psv[:].rearrange("p (h x) -> p h x", h=2),
            QKV_EVAC, None, op0=mybir.AluOpType.mult,
        )
        yield


# --------------------------------------------------------------------------
# attention block: S + exp (backbone), AV + epilogue + OP (fillers)
# --------------------------------------------------------------------------

def emit_s_exp(nc, c, b, qg, h, g2):
    """Two S^T kt tiles + exp -> fp8 P^T tile [128, 2, 512].

    ScalarE path: S pair into one psS [128,2,512] tile, one Exp activation.
    Relay path (every RELAY_MOD-th): S pair into two psM [128,512] tiles
    (keeps the psS ring private to the ScalarE exp chain), DVE-copy to bf16,
    pow(e^c, S) on GPSIMD.
    """
    q0 = qg * 512
    hs = slice(h * 64, (h + 1) * 64)
    name = f"{b}{qg}{h}{g2}"
    idx = c.exp_idx
    c.exp_idx += 1
    pT = c.ptp.tile([128, 2, 512], FP8, tag="pT", name=f"pT{name}")
    relay = idx % RELAY_MOD == RELAY_PHASE

    def s_mm(out, j):
        kt = g2 * 2 + j
        nc.tensor.matmul(
            out,
            c.kt8[b][hs, :, kt * 128:(kt + 1) * 128],
            c.qt8[b][hs, :, q0:q0 + 512],
            start=True, stop=True, perf_mode=DR, tile_position=(h * 64, 0),
        )

    if relay:
        scop = c.work.tile([128, 2, 512], F32, tag="scop", bufs=3,
                           name=f"scop{name}")
        for j in range(2):
            psj = c.psM.tile([128, 512], F32, tag="mm", name=f"psr{name}{j}")
            s_mm(psj[:], j)
            nc.vector.tensor_copy(scop[:, j, :], psj[:])
        nc.gpsimd.tensor_tensor(pT[:], c.econ[:], scop[:], mybir.AluOpType.pow)
    else:
        ps2 = c.psS.tile([128, 2, 512], F32, tag="s2", name=f"ps2_{name}")
        for j in range(2):
            s_mm(ps2[:, j, :], j)
        nc.scalar.activation(pT[:], ps2[:], AF.Exp, scale=EXP_SCALE)
    return pT


def av_epilogue(nc, c, b, qg, h, pts, osts, tail=False):
    """AV + normalize + transpose for block (b,qg,h); one qsub at a time.

    tail=True: spread the copies/scales across ScalarE too (kernel drain,
    ScalarE otherwise idle)."""
    for qsub in range(4):
        po = c.psO.tile([128, 512], F32, tag="acc", name=f"po{b}{qg}{h}{qsub}")
        qs = slice(qsub * 128, (qsub + 1) * 128)
        for g2 in range(8):
            nc.tensor.matmul(
                po[:, 0:65],
                pts[g2][:, :, qs],
                c.v8[b][:, 2 * g2:2 * g2 + 2, h, :],
                start=(g2 == 0), stop=(g2 == 7), perf_mode=DR,
            )
        yield
        recip = c.work.tile([128, 1], F32, tag="recip", bufs=4,
                            name=f"rc{b}{qg}{h}{qsub}")
        nc.vector.reciprocal(recip[:], po[:, 64:65])
        onat = c.work.tile([128, 64], BF16, tag="onat", bufs=4,
                           name=f"on{b}{qg}{h}{qsub}")
        if tail and qsub % 2 == 0:
            nc.scalar.activation(onat[:], po[:, 0:64], AF.Identity,
                                 scale=recip[:])
        else:
            nc.vector.tensor_scalar(
                onat[:], po[:, 0:64], recip[:], None, op0=mybir.AluOpType.mult,
            )
        tr = c.psO.tile([128, 128], BF16, tag="acc", name=f"tr{b}{qg}{h}{qsub}")
        nc.tensor.transpose(tr[0:64, :], onat[:], c.ident[:])
        ost_eng = ("a" if qsub % 2 else "v") if tail else _pick(OST_PAT, c.ost_idx)
        copy_engine(nc, ost_eng)(osts[qsub][0:64, h, :], tr[0:64, :])
        c.ost_idx += 1
        yield


def op_unit(nc, c, b, qg, osts, tail=False):
    """Out-projection for one (b, qg): 4 qsubs x [128, 1024]."""
    for qsub in range(4):
        gq = b * 2048 + qg * 512 + qsub * 128
        ob = c.work.tile([128, 1024], BF16, tag="ob", bufs=3,
                         name=f"ob{b}{qg}{qsub}")
        for n in range(2):
            psy = c.psM.tile([128, 512], F32, tag="mm", name=f"psy{b}{qg}{qsub}{n}")
            nc.tensor.matmul(
                psy[:], osts[qsub][0:64, :, :],
                c.wo_sb[0:64, :, n * 512:(n + 1) * 512],
                start=True, stop=True, perf_mode=DR,
            )
            eng = ("a" if n else "v") if tail else _pick(OP_EVAC_PAT, c.op_idx)
            copy_engine(nc, eng)(ob[:, n * 512:(n + 1) * 512], psy[:])
            c.op_idx += 1
            if tail:
                nc.sync.dma_start(
                    c.out[gq:gq + 128, n * 512:(n + 1) * 512],
                    ob[:, n * 512:(n + 1) * 512])
        if not tail:
            nc.sync.dma_start(c.out[gq:gq + 128, :], ob[:])
        yield


# --------------------------------------------------------------------------

def build_body(tc, xt, wq, wk, wv, bq, wo, out):
    nc = tc.nc
    c = Ctx()
    c.out = out
    c.exp_idx = 0
    c.op_idx = 0
    c.ost_idx = 0
    c.ost_cur = [None] * 4
    c.ost_prev = [None] * 4
    with contextlib.ExitStack() as ctx:
        c.const = ctx.enter_context(tc.tile_pool(name="const", bufs=1))
        c.work = ctx.enter_context(tc.tile_pool(name="work", bufs=3))
        c.ptp = ctx.enter_context(tc.tile_pool(name="ptile", bufs=16))
        # PSUM (8 banks): s2 [128,2,512]f32 x2 = 4, acc(+tr) x2 = 2, mm x2 = 2
        c.psS = ctx.enter_context(tc.tile_pool(name="psS", bufs=2, space="PSUM"))
        c.psO = ctx.enter_context(tc.tile_pool(name="psO", bufs=2, space="PSUM"))
        c.psM = ctx.enter_context(tc.tile_pool(name="psM", bufs=2, space="PSUM"))

        # ---- DMA in consumption order ----
        c.xt_sb = c.const.tile([128, 8, T], FP8, name="xt_sb")

        def load_xt(t0, t1):
            nc.sync.dma_start(c.xt_sb[:, :, t0:t1], xt[:, :, t0:t1])

        c.wk_sb = c.const.tile([128, 8, 128], FP8, name="wk_sb")
        nc.sync.dma_start(c.wk_sb[:], wk[:])
        load_xt(0, 512)
        c.wq_sb = c.const.tile([128, 8, 128], FP8, name="wq_sb")
        nc.sync.dma_start(c.wq_sb[:], wq[:])
        c.bq_sb = c.const.tile([128, 1], F32, name="bq_sb")
        nc.sync.dma_start(c.bq_sb[:], bq[:])
        actwarm = c.work.tile([1, 1], F32, tag="actwarm", bufs=1, name="actwarm")
        nc.scalar.activation(actwarm[:], c.bq_sb[0:1, 0:1], AF.Exp)
        c.wv_sb = c.const.tile([128, 8, 128], FP8, name="wv_sb")
        nc.sync.dma_start(c.wv_sb[:], wv[:])
        load_xt(512, 1024)
        load_xt(1024, 1536)
        load_xt(1536, 2048)
        load_xt(2048, 3072)
        load_xt(3072, 4096)
        c.wo_sb = c.const.tile([64, 2, 1024], FP8, name="wo_sb")
        nc.sync.dma_start(c.wo_sb[:], wo[:])

        c.ident = c.const.tile([128, 128], BF16, name="ident")
        make_identity(nc, c.ident[:])
        c.econ = c.const.tile([128, 2, 512], F32, name="econ")

        # per-batch fp8 operand tensors
        c.qt8 = [c.const.tile([128, 2, S], FP8, name=f"qt8_{b}") for b in range(2)]
        c.kt8 = [c.const.tile([128, 2, S], FP8, name=f"kt8_{b}") for b in range(2)]
        c.v8 = [c.const.tile([128, 16, 2, 65], FP8, name=f"v8_{b}") for b in range(2)]
        # zero the second DR subtile of q/k (0 * finite = 0 in the S matmul).
        # Chunked on Pool in S-consumption order so the first S-pairs don't
        # wait for full-tensor memsets.
        nc.gpsimd.memset(c.kt8[0][:, 1, 0:512], 0.0)
        nc.gpsimd.memset(c.qt8[0][:, 1, 0:512], 0.0)
        nc.gpsimd.memset(c.econ[:], float(np.exp(EXP_SCALE)))
        nc.gpsimd.memset(c.kt8[0][:, 1, 512:2048], 0.0)
        nc.gpsimd.memset(c.qt8[0][:, 1, 512:2048], 0.0)
        for b in range(2):
            nc.vector.memset(c.v8[b][:, :, :, 64:65], 1.0)  # softmax-sum ones
        nc.gpsimd.memset(c.kt8[1][:, 1, :], 0.0)
        nc.gpsimd.memset(c.qt8[1][:, 1, :], 0.0)

        # ---- emission ----
        fill = Filler()
        # prefill: K0, Q0 of batch 0 up-front (S backbone needs them first)
        for _ in k_unit(nc, c, 0, g=0):
            pass
        for _ in q_unit(nc, c, 0, g=0):
            pass
        k_gens = {0: [], 1: []}
        q_gens = {}
        v_gens = {}
        for g in range(1, 4):
            k_gens[0].append(fill.add(k_unit(nc, c, 0, g=g)))
        v_gens[0] = fill.add(v_units(nc, c, 0))
        for g in range(1, 4):
            q_gens[(0, g)] = fill.add(q_unit(nc, c, 0, g=g))
        for g in range(4):
            k_gens[1].append(fill.add(k_unit(nc, c, 1, g=g)))
        q_gens[(1, 0)] = fill.add(q_unit(nc, c, 1, g=0))
        v_gens[1] = fill.add(v_units(nc, c, 1))
        for g in range(1, 4):
            q_gens[(1, g)] = fill.add(q_unit(nc, c, 1, g=g))

        blocks = [(b, qg, h) for b in range(2) for qg in range(4) for h in range(2)]
        osts_map = {}

        def osts_for(b, qg):
            if (b, qg) not in osts_map:
                osts_map[(b, qg)] = [
                    c.work.tile([64, 2, 128], FP8, tag=f"osT{q}", bufs=2,
                                name=f"osT{b}{qg}{q}")
                    for q in range(4)
                ]
            return osts_map[(b, qg)]

        pts_prev = None
        prev_blk = None
        for n, (b, qg, h) in enumerate(blocks):
            # emission-order prerequisites: the epilogue of prev_blk reads
            # v8[prev_b]; this block's S matmuls read kt8/qt8 slices.
            if prev_blk is not None:
                fill.drain_until(v_gens[prev_blk[0]])
            for kg in k_gens[b]:
                fill.drain_until(kg)
            if (b, qg) in q_gens:
                fill.drain_until(q_gens[(b, qg)])
            pts = []
            if prev_blk is not None:
                fill.add_front(av_epilogue(nc, c, *prev_blk, pts_prev,
                                           osts_for(prev_blk[0], prev_blk[1])))
            for g2 in range(8):
                pts.append(emit_s_exp(nc, c, b, qg, h, g2))
                fill.pull(2)
            if prev_blk is not None and prev_blk[2] == 1:
                # both heads of (prev b, prev qg) done once its epilogue runs
                fill.add(op_unit(nc, c, prev_blk[0], prev_blk[1],
                                 osts_for(prev_blk[0], prev_blk[1])))
            pts_prev = pts
            prev_blk = (b, qg, h)
        fill.add_front(av_epilogue(nc, c, *prev_blk, pts_prev,
                                   osts_for(prev_blk[0], prev_blk[1]),
                                   tail=True))
        fill.add(op_unit(nc, c, prev_blk[0], prev_blk[1],
                         osts_for(prev_blk[0], prev_blk[1]), tail=True))
        fill.drain()


def build_program():
    key = (RELAY_MOD, RELAY_PHASE, OP_EVAC_PAT, QK_EVAC_PAT, V_EVAC_PAT,
           ONAT_PAT, OST_PAT)
    if key in _program_cache:
        return _program_cache[key]
    nc = bacc.Bacc("TRN2", target_bir_lowering=False, debug=False)
    xt = nc.dram_tensor("xt", [128, 8, T], FP8, kind="ExternalInput").ap()
    wq = nc.dram_tensor("wq", [128, 8, 128], FP8, kind="ExternalInput").ap()
    wk = nc.dram_tensor("wk", [128, 8, 128], FP8, kind="ExternalInput").ap()
    wv = nc.dram_tensor("wv", [128, 8, 128], FP8, kind="ExternalInput").ap()
    bq = nc.dram_tensor("bq", [128, 1], F32, kind="ExternalInput").ap()
    wo = nc.dram_tensor("wo", [64, 2, 1024], FP8, kind="ExternalInput").ap()
    out = nc.dram_tensor("out", [T, H], BF16, kind="ExternalOutput").ap()
    with tile.TileContext(nc) as tc:
        build_body(tc, xt, wq, wk, wv, bq, wo, out)
    nc.compile()
    _program_cache[key] = nc
    return nc


def make_in_maps(x, w_qkv, b_qkv, w_out):
    x = np.asarray(x, dtype=np.float32)
    w_qkv = np.asarray(w_qkv, dtype=np.float32)
    b_qkv = np.asarray(b_qkv, dtype=np.float32)
    w_out = np.asarray(w_out, dtype=np.float32)

    # x^T [H, T] scaled by 8, in [128, 8, T] layout (hidden ktile on dim1)
    xt = np.ascontiguousarray(
        (x.reshape(T, H).T * 8.0).reshape(8, 128, T).transpose(1, 0, 2)
    ).astype(f8np)

    def prep_w(w):
        # [1024 hidden, 128 cols] -> [128 part, 8 ktile, 128 col], *64
        return np.ascontiguousarray(
            (w * 64.0).reshape(8, 128, 128).transpose(1, 0, 2)
        ).astype(f8np)

    in_maps = []
    for cc in range(N_CORES):
        sl = slice(cc * 128, (cc + 1) * 128)
        wo_c = np.ascontiguousarray(
            (w_out[sl, :] * 64.0).reshape(2, 64, H).transpose(1, 0, 2)
        ).astype(f8np)
        in_maps.append({
            "xt": xt,
            "wq": prep_w(w_qkv[:, sl]),
            "wk": prep_w(w_qkv[:, H + cc * 128:H + (cc + 1) * 128]),
            "wv": prep_w(w_qkv[:, 2 * H + cc * 128:2 * H + (cc + 1) * 128]),
            "bq": (b_qkv[sl] * 16.0).astype(np.float32).reshape(128, 1),
            "wo": wo_c,
        })
    return in_maps


def finalize(results, b_qkv, b_out, w_out):
    b_qkv = np.asarray(b_qkv, dtype=np.float32)
    b_out = np.asarray(b_out, dtype=np.float32)
    w_out = np.asarray(w_out, dtype=np.float32)
    acc = np.zeros((T, H), np.float32)
    for r in results:
        acc += np.asarray(r["out"], dtype=np.float32)
    acc /= OUT_DIV
    corr = b_out + b_qkv[2 * H:] @ w_out
    return (acc + corr).reshape(B, S, H).astype(np.float32)


def kernel(x, w_qkv, b_qkv, w_out, b_out):
    import os

    os.environ["BASS_NEVER_TRACE"] = "1"
    nc = build_program()
    in_maps = make_in_maps(x, w_qkv, b_qkv, w_out)
    res = run_bass_kernel_spmd(nc, in_maps, list(range(N_CORES)))
    return finalize(res.results, b_qkv, b_out, w_out)
